# revision 31
# baseline (speedup 1.0000x reference)
"""Causal multi-head attention block (B=2, T=2048, C=1024, H=16) on 8 TRN2
NeuronCores.

Sharding: 2D tensor parallel — core r owns batch b = r//4 and head quad
g = r%4 (heads 4g..4g+3, feature slice [256g, 256g+256)). Each core
projects q/k/v for its 256 features over its batch's 2048 tokens
(x^T replicated per batch), runs causal attention for its 4 heads, then
computes a PARTIAL output projection out_partial = Wo[:, 256g:256g+256]
@ y_local — full 1024 output rows, contracting only the local features.
The 4 cores of a batch ReduceScatter(add) the partials so core g ends
with output rows [256g, 256g+256) — 3 MB of wire per core instead of
the ~7 MB an 8-way y-AllGather moves, and the O-projection matmuls are
collective-independent PE filler instead of tail work.

On-device everything is feature-major (transposed) so the TensorEngine
contraction axis sits on SBUF partitions and the softmax denominator
arrives via a ones-column appended to V:

  qT/kT [128, 2, 2048] = W_shard @ x^T           (2 feature tiles)
  v_sb[128tok, kt, h, 0:64] = x_tile^T @ Wv      (v built NATURALLY per
                                                  token tile — no
                                                  transpose pass)
  ST tile [128k, 512q] = kT[64h slice].T @ qT    (contract d=64)
  PT = exp(ST * 1/sqrt(d))                       (no max-subtraction:
                                                  logits ~N(0,1))
  causal mask: PT[diag 128-block] *= tri (upper-triangular 0/1 bf16,
      vector multiply) — cheaper than injecting -1e9 into PSUM via
      matmul, which cost an ident LDWEIGHTS + matmul per diagonal tile
  yT [65, 512] += [v | 1].T @ PT                 (row 64 = softmax denom)
  ych[*, 512]  = yT[0:64] * bcast(recip(yT[64]))
  po [128rt, 512] += WoT[ci, rt] @ ych[:, ci]    (8 row tiles × 2 ci)

Performance structure (from perfetto/HAM analysis): the PE idle-activity
throttle (HAM) halves the clock for any window containing idle time — a
dense-matmul microbench holds 2.4 GHz while the v1 kernel averaged
~0.63×. So the whole kernel is ONE software-pipelined job stream:
score-pair jobs and filler GEMM jobs (projection chunks, partial-O
groups, v tiles) are interleaved, with each pair's PV matmuls deferred
two jobs so the PE never waits on the exp of the pair it just scored.
ReduceScatter chunks fire per 512-token block as soon as the block's
partial-O lands in DRAM; rs_out -> out DMAs all run at program end
(a DMA after an RS trigger on a compute queue would block that queue on
RS completion and stall the pipeline).

Inputs are bf16 (host-side cast); accumulation is f32 in PSUM; the
output shard is written bf16 and upcast to f32 on the host.
"""

import numpy as np
import ml_dtypes

import concourse.bacc as bacc
import concourse.mybir as mybir
import concourse.tile as tile
from concourse.bass_utils import run_bass_kernel_spmd
from concourse.masks import make_identity

N_CORES = 8
B, T, C, H = 2, 2048, 1024, 16
D = 64                # head dim
GR = 4                # head-group cores per batch
HL = 4                # heads per core
DL = HL * D           # local feature dim = 256
MT = DL // 128        # feature tiles per core = 2
TL = T                # local tokens per core = its batch's 2048
P = 128
NCH = C // P          # 8 contraction chunks for q/k/v projections
QCH = 512             # q-chunk / token chunk
NQC = TL // QCH       # 4 chunks
NKT = TL // P         # 16 k-tiles
NRT = C // P          # 8 output row tiles for partial O-proj
SCALE = 1.0 / np.sqrt(D)
LA = 2                # job-stream lookahead (score pairs ahead of PV)

BF = mybir.dt.bfloat16
F32 = mybir.dt.float32
AF = mybir.ActivationFunctionType

RGROUPS = [[0, 1, 2, 3], [4, 5, 6, 7]]


def build_graph():
    nc = bacc.Bacc("TRN2", target_bir_lowering=False, debug=False)

    xT = nc.dram_tensor("xT", [C, TL], BF, kind="ExternalInput")
    # qkv shards [p, w, ci, m(256)] then woT [p, ci2, rt*128]
    WQKV = 3 * NCH * DL
    WO = MT * C
    wall = nc.dram_tensor("wall", [P, WQKV + WO], BF, kind="ExternalInput")
    out = nc.dram_tensor("out", [DL, TL], BF, kind="ExternalOutput")

    with tile.TileContext(nc) as tc:
        with (
            tc.tile_pool(name="sb", bufs=1) as sb,
            tc.tile_pool(name="ps", bufs=1, space="PSUM") as ps,
            tc.tile_pool(name="dram", bufs=1, space="DRAM") as dram,
        ):
            # ---- loads ----
            # separate tiles per weight and per x-chunk group so a
            # consumer only waits on ITS producer DMA, not all of them
            wq_full = sb.tile([P, NCH * DL], BF, name="wq_full")
            wk_full = sb.tile([P, NCH * DL], BF, name="wk_full")
            wv_full = sb.tile([P, NCH * DL], BF, name="wv_full")
            wo_full = sb.tile([P, MT * C], BF, name="wo_full")
            # x split: first 512 tokens per ci land fast, rest batched
            x_head = sb.tile([P, NCH, QCH], BF, name="x_head")
            x_tail = sb.tile([P, NCH, TL - QCH], BF, name="x_tail")

            ENGS3 = (nc.scalar, nc.sync, nc.gpsimd)
            nc.scalar.dma_start(wq_full[:], wall[:, 0:NCH * DL])
            for ci in range(NCH):
                ENGS3[ci % 3].dma_start(
                    x_head[:, ci, :], xT[ci * P:(ci + 1) * P, 0:QCH]
                )
            nc.sync.dma_start(wk_full[:], wall[:, NCH * DL:2 * NCH * DL])
            nc.gpsimd.dma_start(wv_full[:],
                                wall[:, 2 * NCH * DL:3 * NCH * DL])
            for ci in range(NCH):
                ENGS3[ci % 3].dma_start(
                    x_tail[:, ci, :], xT[ci * P:(ci + 1) * P, QCH:TL]
                )
            nc.scalar.dma_start(wo_full[:], wall[:, WQKV:])

            def xap(ci, t0, t1):
                # token-range view across the head/tail split tiles
                assert t0 >= QCH or t1 <= QCH
                if t1 <= QCH:
                    return x_head[:, ci, t0:t1]
                return x_tail[:, ci, t0 - QCH:t1 - QCH]

            wq_sb = wq_full[:].rearrange("p (a m) -> p a m", a=NCH)
            wk_sb = wk_full[:].rearrange("p (a m) -> p a m", a=NCH)
            wv_sb = wv_full[:].rearrange("p (a m) -> p a m", a=NCH)
            wo_sb = wo_full[:].rearrange("p (a r) -> p a r", a=MT)

            # warm the collective stream: the first CC op on a cold
            # stream pays ~11us of setup before data moves
            cc_warm_in = dram.tile([GR, 16], BF, name="cc_warm_in")
            cc_warm_out = dram.tile([1, 16], BF, name="cc_warm_out")
            nc.gpsimd.collective_compute(
                "ReduceScatter", mybir.AluOpType.add,
                replica_groups=RGROUPS,
                ins=[cc_warm_in[:]], outs=[cc_warm_out[:]],
            )

            ident = sb.tile([P, P], BF, name="ident")
            make_identity(nc, ident)
            # upper-triangular (q >= k) 0/1 mask for diagonal blocks
            tri = sb.tile([P, P], BF, name="tri")
            nc.gpsimd.memset(tri[:], 1.0)
            nc.gpsimd.affine_select(
                out=tri[:], in_=tri[:],
                compare_op=mybir.AluOpType.is_ge,
                fill=0.0, base=0, channel_multiplier=-1, pattern=[[1, P]],
            )
            wsrc = sb.tile([P, QCH], BF, name="wsrc")
            nc.vector.memset(wsrc[:], 0.5)

            def keepwarm(n):
                for _ in range(n):
                    wdst = ps.tile([P, QCH], F32, tag="st", bufs=3,
                                   name="wdst")
                    nc.tensor.matmul(wdst[:], ident[:], wsrc[:],
                                     start=True, stop=True)

            keepwarm(20)

            qT_sb = sb.tile([P, MT, TL], BF, name="qT_sb")
            kT_sb = sb.tile([P, MT, TL], BF, name="kT_sb")
            # v natural layout, per 128-token tile per head: [64 v | 1]
            v_sb = sb.tile([P, NKT, HL, D + 1], BF, name="v_sb")
            nc.gpsimd.memset(v_sb[:], 1.0)

            # blocks: (q0, qw, nkt). The last 512-token block is split
            # into two 256-wide halves so its first RS overlaps the
            # remaining attention and the tail RS is half-size.
            BLOCKS = [
                (0, QCH, 4), (QCH, QCH, 8), (2 * QCH, QCH, 12),
                (3 * QCH, QCH // 2, 14), (3 * QCH + QCH // 2, QCH // 2, 16),
            ]
            NBLK = len(BLOCKS)

            po_dram = [
                dram.tile([C, qw], BF, name=f"po_dram{c}")
                for c, (q0, qw, nkt) in enumerate(BLOCKS)
            ]
            rs_out = [
                dram.tile([DL, qw], BF, name=f"rs_out{c}")
                for c, (q0, qw, nkt) in enumerate(BLOCKS)
            ]

            # ---------- job bodies ----------
            def proj_one(tch, wsb, mt, dst):
                # single 512-wide chunk (used early: evicts ASAP)
                t0 = tch * QCH
                pj = ps.tile([P, QCH], F32, tag="st", bufs=3, name="pj")
                for ci in range(NCH):
                    nc.tensor.matmul(
                        pj[:], wsb[:, ci, mt * P:(mt + 1) * P],
                        xap(ci, t0, t0 + QCH),
                        start=(ci == 0), stop=(ci == NCH - 1),
                    )
                nc.vector.tensor_copy(dst[:, mt, t0:t0 + QCH], pj[:])

            def proj_pair(pch, wsb, mt, dst):
                # two 512-wide accumulation groups, one 1024-wide evict
                t0 = pch * 2 * QCH
                pj = ps.tile([P, 2 * QCH], F32, tag="st", bufs=3, name="pj")
                for half in range(2):
                    h0 = t0 + half * QCH
                    for ci in range(NCH):
                        nc.tensor.matmul(
                            pj[:, half * QCH:(half + 1) * QCH],
                            wsb[:, ci, mt * P:(mt + 1) * P],
                            xap(ci, h0, h0 + QCH),
                            start=(ci == 0), stop=(ci == NCH - 1),
                        )
                nc.vector.tensor_copy(
                    dst[:, mt, t0:t0 + 2 * QCH], pj[:]
                )

            def v_tile(t16):
                # v_nat [128 tok, 256 feat] = x_tile^T @ Wv
                vps = ps.tile([P, DL], F32, tag="st", bufs=3, name="vps")
                for ci in range(NCH):
                    nc.tensor.matmul(
                        vps[:], xap(ci, t16 * P, (t16 + 1) * P),
                        wv_sb[:, ci, :],
                        start=(ci == 0), stop=(ci == NCH - 1),
                    )
                nc.vector.tensor_copy(
                    v_sb[:, t16, :, 0:D],
                    vps[:].rearrange("p (h x) -> p h x", h=HL),
                )

            ych_tiles = {}
            yt_tiles = {}

            def scores(blk, h, pr):
                """Scores + exp (+ causal mask) for one k-tile pair."""
                q0, qw, nkt = BLOCKS[blk]
                mt, hh = h // 2, h % 2
                rsl = slice(hh * D, (hh + 1) * D)
                st = ps.tile([P, 2 * qw], F32, tag="st", bufs=3, name="st")
                pt = sb.tile([P, 2 * qw], BF, tag="pt", bufs=5, name="pt")
                diag = []
                for half in range(2):
                    kt = 2 * pr + half
                    qv = max(kt * P - q0, 0)
                    nc.tensor.matmul(
                        st[:, half * qw + qv:(half + 1) * qw],
                        kT_sb[rsl, mt, kt * P:(kt + 1) * P],
                        qT_sb[rsl, mt, q0 + qv:q0 + qw],
                        start=True, stop=True,
                    )
                    if kt * P >= q0:
                        diag.append(half * qw + qv)
                qv0 = max(2 * pr * P - q0, 0)
                nc.scalar.activation(
                    pt[:, qv0:], st[:, qv0:], AF.Exp, scale=float(SCALE)
                )
                for c0 in diag:
                    # on gpsimd: vector's eviction backlog would gate
                    # the PV matmuls behind these
                    nc.gpsimd.tensor_mul(
                        pt[:, c0:c0 + P], pt[:, c0:c0 + P], tri[:]
                    )
                return pt

            def pv(blk, h, pr, pt):
                q0, qw, nkt = BLOCKS[blk]
                yt = yt_tiles.get((blk, h))
                if yt is None:
                    yt = yt_tiles[(blk, h)] = ps.tile(
                        [D + 1, qw], F32, tag="yt", bufs=2, name="yt"
                    )
                for half in range(2):
                    kt = 2 * pr + half
                    qv = max(kt * P - q0, 0)
                    nc.tensor.matmul(
                        yt[:, qv:qw],
                        v_sb[:, kt, h, :],
                        pt[:, half * qw + qv:(half + 1) * qw],
                        start=(kt == 0), stop=(kt == nkt - 1),
                    )

            def evict(blk, h):
                q0, qw, nkt = BLOCKS[blk]
                yt = yt_tiles.pop((blk, h))
                den = sb.tile([1, qw], F32, tag="den", bufs=4, name="den")
                nc.vector.tensor_copy(den[:], yt[D:D + 1, :])
                bc = sb.tile([D, qw], F32, tag="bc", bufs=3, name="bc")
                nc.gpsimd.partition_broadcast(bc[:], den[:])
                rcp = sb.tile([D, qw], F32, tag="rcp", bufs=3, name="rcp")
                scr = sb.tile([D, qw], F32, tag="scr", bufs=3, name="scr")
                nc.vector.reciprocal_approx_accurate(
                    rcp[:], bc[:], scratch=scr[:]
                )
                mt, hh = h // 2, h % 2
                ych = ych_tiles[blk]
                nc.vector.tensor_mul(
                    ych[hh * D:(hh + 1) * D, mt, :], yt[0:D, :], rcp[:]
                )

            def po_group(c):
                """Partial O-proj for block c -> DRAM -> RS trigger."""
                q0, qw, nkt = BLOCKS[c]
                ych = ych_tiles.pop(c)
                ob = sb.tile([P, NRT, qw], BF, tag="ob", bufs=2, name="ob")
                for rt in range(NRT):
                    po = ps.tile([P, qw], F32, tag="st", bufs=3, name="po")
                    for ci in range(MT):
                        nc.tensor.matmul(
                            po[:], wo_sb[:, ci, rt * P:(rt + 1) * P],
                            ych[:, ci, :],
                            start=(ci == 0), stop=(ci == MT - 1),
                        )
                    nc.vector.tensor_copy(ob[:, rt, :], po[:])
                dview = po_dram[c][:].rearrange("(r p) t -> p r t", p=P)
                nc.sync.dma_start(dview, ob[:])
                nc.gpsimd.collective_compute(
                    "ReduceScatter",
                    mybir.AluOpType.add,
                    replica_groups=RGROUPS,
                    ins=[po_dram[c][:]],
                    outs=[rs_out[c][:]],
                )

            # ---------- job stream ----------
            # each job: (phase1, phase2) — phase2 runs LA jobs later
            jobs = []

            def add_filler(fn, *args):
                jobs.append((lambda a=args: fn(*a), None))

            def add_pair(blk, h, pr, last):
                def p1(a=(blk, h, pr)):
                    return scores(*a)

                def p2(pt, a=(blk, h, pr), last=last):
                    pv(*a, pt)
                    if last:
                        evict(a[0], a[1])
                jobs.append((p1, p2))

            # chunk 0 projections (immediate: block 0 needs them; solo
            # 512-wide so they only gate on the fast x_head DMAs)
            for wsb, dst in ((wq_sb, qT_sb), (wk_sb, kT_sb)):
                for mt in range(MT):
                    add_filler(proj_one, 0, wsb, mt, dst)
            for t16 in range(4):
                add_filler(v_tile, t16)

            # blocks with fillers woven in
            for blk, (q0, qw, nkt) in enumerate(BLOCKS):
                npr = nkt // 2
                for h in range(HL):
                    for pr in range(npr):
                        add_pair(blk, h, pr, pr == npr - 1)
                    # weave fillers after each head
                    if blk == 0:
                        if h == 0:
                            for wsb, dst in ((wq_sb, qT_sb), (wk_sb, kT_sb)):
                                for mt in range(MT):
                                    add_filler(proj_one, 1, wsb, mt, dst)
                        elif h == 1:
                            for t16 in range(4, 8):
                                add_filler(v_tile, t16)
                        elif h == 2:
                            for t16 in range(8, 12):
                                add_filler(v_tile, t16)
                        else:
                            for t16 in range(12, 16):
                                add_filler(v_tile, t16)
                    elif blk == 1:
                        if h == 0:
                            add_filler(po_group, 0)
                        elif h == 1:
                            for mt in range(MT):
                                add_filler(proj_pair, 1, wq_sb, mt, qT_sb)
                        elif h == 2:
                            for mt in range(MT):
                                add_filler(proj_pair, 1, wk_sb, mt, kT_sb)
                    elif blk > 1 and h == 0:
                        add_filler(po_group, blk - 1)
            # flush the pipeline (phase2 lags by LA) before the last po
            for _ in range(LA):
                add_filler(keepwarm, 1)
            add_filler(po_group, NBLK - 1)

            # ---------- software-pipelined emission ----------
            for blk, (q0, qw, nkt) in enumerate(BLOCKS):
                ych_tiles[blk] = sb.tile([P, MT, qw], BF, tag="ych",
                                         bufs=2, name="ych")

            pending = []
            for i in range(len(jobs) + LA):
                if i < len(jobs):
                    p1, p2 = jobs[i]
                    r = p1()
                    pending.append((p2, r))
                if i >= LA:
                    p2, r = pending[i - LA]
                    if p2 is not None:
                        p2(r)

            engs = [nc.sync, nc.gpsimd]
            for c, (q0, qw, nkt) in enumerate(BLOCKS):
                engs[c % 2].dma_start(out[:, q0:q0 + qw], rs_out[c][:])

    nc.finalize()
    return nc


# pv() needs yt allocated; allocate inside pv via yt_tiles guard
_GRAPH = None


def _get_graph():
    global _GRAPH
    if _GRAPH is None:
        _GRAPH = build_graph()
    return _GRAPH


def prepare_in_maps(x, Wq, Wk, Wv, Wo):
    x = np.asarray(x, np.float32)
    Wq = np.asarray(Wq, np.float32)
    Wk = np.asarray(Wk, np.float32)
    Wv = np.asarray(Wv, np.float32)
    Wo = np.asarray(Wo, np.float32)

    bf = ml_dtypes.bfloat16
    xTh = [np.ascontiguousarray(x[b].T).astype(bf) for b in range(B)]
    in_maps = []
    for r in range(N_CORES):
        b, g = r // GR, r % GR
        sl = slice(g * DL, (g + 1) * DL)
        wqkv = np.empty((P, 3, NCH, DL), np.float32)
        for w, W in enumerate((Wq, Wk, Wv)):
            wqkv[:, w] = W[sl].T.reshape(NCH, P, DL).transpose(1, 0, 2)
        wo = np.ascontiguousarray(Wo[:, sl].T)  # [DL, C] = lhsT
        woT = wo.reshape(MT, P, C).transpose(1, 0, 2)  # [p, ci, C]
        wall = np.concatenate(
            [wqkv.reshape(P, 3 * NCH * DL), woT.reshape(P, MT * C)], axis=1
        )
        in_maps.append({
            "xT": xTh[b],
            "wall": np.ascontiguousarray(wall).astype(bf),
        })
    return in_maps


def assemble_output(results):
    outT = np.empty((B, C, TL), np.float32)
    for r in range(N_CORES):
        b, g = r // GR, r % GR
        outT[b, g * DL:(g + 1) * DL] = np.asarray(
            results[r]["out"], np.float32
        )
    return np.ascontiguousarray(outT.transpose(0, 2, 1))  # [B, T, C]


def kernel(x, Wq, Wk, Wv, Wo):
    nc = _get_graph()
    in_maps = prepare_in_maps(x, Wq, Wk, Wv, Wo)
    res = run_bass_kernel_spmd(nc, in_maps, core_ids=list(range(N_CORES)))
    return assemble_output(res.results)


# revision 32
# speedup vs baseline: 1.5201x; 1.5201x over previous
"""Causal multi-head attention block (B=2, T=2048, C=1024, H=16) on 8 TRN2
NeuronCores.

Sharding: 2D tensor parallel — core r owns batch b = r//4 and head quad
g = r%4 (heads 4g..4g+3, feature slice [256g, 256g+256)). Each core
projects q/k/v for its 256 features over its batch's 2048 tokens
(x^T replicated per batch), runs causal attention for its 4 heads, then
computes a PARTIAL output projection out_partial = Wo[:, 256g:256g+256]
@ y_local — full 1024 output rows, contracting only the local features.
The 4 cores of a batch ReduceScatter(add) the partials so core g ends
with output rows [256g, 256g+256) — 3 MB of wire per core instead of
the ~7 MB an 8-way y-AllGather moves, and the O-projection matmuls are
collective-independent PE filler instead of tail work.

On-device everything is feature-major (transposed) so the TensorEngine
contraction axis sits on SBUF partitions and the softmax denominator
arrives via a ones-column appended to V:

  qT/kT [128, 2, 2048] = W_shard @ x^T           (2 feature tiles)
  v_sb[128tok, kt, h, 0:64] = x_tile^T @ Wv      (v built NATURALLY per
                                                  token tile — no
                                                  transpose pass)
  ST tile [128k, 512q] = kT[64h slice].T @ qT    (contract d=64)
  PT = exp(ST * 1/sqrt(d))                       (no max-subtraction:
                                                  logits ~N(0,1))
  causal mask: PT[diag 128-block] *= tri (upper-triangular 0/1 bf16,
      vector multiply) — cheaper than injecting -1e9 into PSUM via
      matmul, which cost an ident LDWEIGHTS + matmul per diagonal tile
  yT [65, 512] += [v | 1].T @ PT                 (row 64 = softmax denom)
  ych[*, 512]  = yT[0:64] * bcast(recip(yT[64]))
  po [128rt, 512] += WoT[ci, rt] @ ych[:, ci]    (8 row tiles × 2 ci)

Performance structure (from perfetto/HAM analysis): the PE idle-activity
throttle (HAM) halves the clock for any window containing idle time — a
dense-matmul microbench holds 2.4 GHz while the v1 kernel averaged
~0.63×. So the whole kernel is ONE software-pipelined job stream:
score-pair jobs and filler GEMM jobs (projection chunks, partial-O
groups, v tiles) are interleaved, with each pair's PV matmuls deferred
two jobs so the PE never waits on the exp of the pair it just scored.
ReduceScatter chunks fire per 512-token block as soon as the block's
partial-O lands in DRAM; rs_out -> out DMAs all run at program end
(a DMA after an RS trigger on a compute queue would block that queue on
RS completion and stall the pipeline).

Inputs are bf16 (host-side cast); accumulation is f32 in PSUM; the
output shard is written bf16 and upcast to f32 on the host.
"""

import numpy as np
import ml_dtypes

import concourse.bacc as bacc
import concourse.mybir as mybir
import concourse.tile as tile
from concourse.bass_utils import run_bass_kernel_spmd
from concourse.masks import make_identity

N_CORES = 8
B, T, C, H = 2, 2048, 1024, 16
D = 64                # head dim
GR = 4                # head-group cores per batch
HL = 4                # heads per core
DL = HL * D           # local feature dim = 256
MT = DL // 128        # feature tiles per core = 2
TL = T                # local tokens per core = its batch's 2048
P = 128
NCH = C // P          # 8 contraction chunks for q/k/v projections
QCH = 512             # q-chunk / token chunk
NQC = TL // QCH       # 4 chunks
NKT = TL // P         # 16 k-tiles
NRT = C // P          # 8 output row tiles for partial O-proj
SCALE = 1.0 / np.sqrt(D)
LA = 2                # job-stream lookahead (score pairs ahead of PV)

BF = mybir.dt.bfloat16
F32 = mybir.dt.float32
AF = mybir.ActivationFunctionType

RGROUPS = [[0, 1, 2, 3], [4, 5, 6, 7]]


def build_graph():
    nc = bacc.Bacc("TRN2", target_bir_lowering=False, debug=False)

    xT = nc.dram_tensor("xT", [C, TL], BF, kind="ExternalInput")
    # qkv shards [p, w, ci, m(256)] then woT [p, ci2, rt*128]
    WQKV = 3 * NCH * DL
    WO = MT * C
    wall = nc.dram_tensor("wall", [P, WQKV + WO], BF, kind="ExternalInput")
    out = nc.dram_tensor("out", [DL, TL], BF, kind="ExternalOutput")

    with tile.TileContext(nc) as tc:
        with (
            tc.tile_pool(name="sb", bufs=1) as sb,
            tc.tile_pool(name="ps", bufs=1, space="PSUM") as ps,
            tc.tile_pool(name="dram", bufs=1, space="DRAM") as dram,
        ):
            # ---- loads ----
            # separate tiles per weight and per x-chunk group so a
            # consumer only waits on ITS producer DMA, not all of them
            wq_full = sb.tile([P, NCH * DL], BF, name="wq_full")
            wk_full = sb.tile([P, NCH * DL], BF, name="wk_full")
            wv_full = sb.tile([P, NCH * DL], BF, name="wv_full")
            wo_full = sb.tile([P, MT * C], BF, name="wo_full")
            # x split: first 512 tokens per ci land fast, rest batched
            x_head = sb.tile([P, NCH, QCH], BF, name="x_head")
            x_tail = sb.tile([P, NCH, TL - QCH], BF, name="x_tail")

            ENGS3 = (nc.scalar, nc.sync, nc.gpsimd)
            nc.scalar.dma_start(wq_full[:], wall[:, 0:NCH * DL])
            for ci in range(NCH):
                ENGS3[ci % 3].dma_start(
                    x_head[:, ci, :], xT[ci * P:(ci + 1) * P, 0:QCH]
                )
            nc.sync.dma_start(wk_full[:], wall[:, NCH * DL:2 * NCH * DL])
            nc.gpsimd.dma_start(wv_full[:],
                                wall[:, 2 * NCH * DL:3 * NCH * DL])
            for ci in range(NCH):
                ENGS3[ci % 3].dma_start(
                    x_tail[:, ci, :], xT[ci * P:(ci + 1) * P, QCH:TL]
                )
            nc.scalar.dma_start(wo_full[:], wall[:, WQKV:])

            def xap(ci, t0, t1):
                # token-range view across the head/tail split tiles
                assert t0 >= QCH or t1 <= QCH
                if t1 <= QCH:
                    return x_head[:, ci, t0:t1]
                return x_tail[:, ci, t0 - QCH:t1 - QCH]

            wq_sb = wq_full[:].rearrange("p (a m) -> p a m", a=NCH)
            wk_sb = wk_full[:].rearrange("p (a m) -> p a m", a=NCH)
            wv_sb = wv_full[:].rearrange("p (a m) -> p a m", a=NCH)
            wo_sb = wo_full[:].rearrange("p (a r) -> p a r", a=MT)

            # warm the collective stream: the first CC op on a cold
            # stream pays ~11us of setup before data moves
            cc_warm_in = dram.tile([GR, 16], BF, name="cc_warm_in")
            cc_warm_out = dram.tile([1, 16], BF, name="cc_warm_out")
            nc.gpsimd.collective_compute(
                "ReduceScatter", mybir.AluOpType.add,
                replica_groups=RGROUPS,
                ins=[cc_warm_in[:]], outs=[cc_warm_out[:]],
            )

            ident = sb.tile([P, P], BF, name="ident")
            make_identity(nc, ident)
            # upper-triangular (q >= k) 0/1 mask for diagonal blocks
            tri = sb.tile([P, P], BF, name="tri")
            nc.gpsimd.memset(tri[:], 1.0)
            nc.gpsimd.affine_select(
                out=tri[:], in_=tri[:],
                compare_op=mybir.AluOpType.is_ge,
                fill=0.0, base=0, channel_multiplier=-1, pattern=[[1, P]],
            )
            wsrc = sb.tile([P, QCH], BF, name="wsrc")
            nc.vector.memset(wsrc[:], 0.5)

            def keepwarm(n):
                for _ in range(n):
                    wdst = ps.tile([P, QCH], F32, tag="st", bufs=3,
                                   name="wdst")
                    nc.tensor.matmul(wdst[:], ident[:], wsrc[:],
                                     start=True, stop=True)

            keepwarm(20)

            qT_sb = sb.tile([P, MT, TL], BF, name="qT_sb")
            kT_sb = sb.tile([P, MT, TL], BF, name="kT_sb")
            # v natural layout, per 128-token tile per head: [64 v | 1]
            v_sb = sb.tile([P, NKT, HL, D + 1], BF, name="v_sb")
            nc.gpsimd.memset(v_sb[:], 1.0)

            # blocks: (q0, qw, nkt). The last 512-token block is split
            # into two 256-wide halves so its first RS overlaps the
            # remaining attention and the tail RS is half-size.
            BLOCKS = [
                (0, QCH, 4), (QCH, QCH, 8), (2 * QCH, QCH, 12),
                (3 * QCH, QCH // 2, 14), (3 * QCH + QCH // 2, QCH // 2, 16),
            ]
            NBLK = len(BLOCKS)

            po_dram = [
                dram.tile([C, qw], BF, name=f"po_dram{c}")
                for c, (q0, qw, nkt) in enumerate(BLOCKS)
            ]
            rs_out = [
                dram.tile([DL, qw], BF, name=f"rs_out{c}")
                for c, (q0, qw, nkt) in enumerate(BLOCKS)
            ]

            # ---------- job bodies ----------
            def proj_one(tch, wsb, mt, dst):
                # single 512-wide chunk (used early: evicts ASAP)
                t0 = tch * QCH
                pj = ps.tile([P, QCH], F32, tag="st", bufs=3, name="pj")
                for ci in range(NCH):
                    nc.tensor.matmul(
                        pj[:], wsb[:, ci, mt * P:(mt + 1) * P],
                        xap(ci, t0, t0 + QCH),
                        start=(ci == 0), stop=(ci == NCH - 1),
                    )
                nc.vector.tensor_copy(dst[:, mt, t0:t0 + QCH], pj[:])

            def proj_pair(pch, wsb, mt, dst):
                # two 512-wide accumulation groups, one 1024-wide evict
                t0 = pch * 2 * QCH
                pj = ps.tile([P, 2 * QCH], F32, tag="st", bufs=3, name="pj")
                for half in range(2):
                    h0 = t0 + half * QCH
                    for ci in range(NCH):
                        nc.tensor.matmul(
                            pj[:, half * QCH:(half + 1) * QCH],
                            wsb[:, ci, mt * P:(mt + 1) * P],
                            xap(ci, h0, h0 + QCH),
                            start=(ci == 0), stop=(ci == NCH - 1),
                        )
                nc.vector.tensor_copy(
                    dst[:, mt, t0:t0 + 2 * QCH], pj[:]
                )

            def v_tile(t16):
                # v_nat [128 tok, 256 feat] = x_tile^T @ Wv
                vps = ps.tile([P, DL], F32, tag="st", bufs=3, name="vps")
                for ci in range(NCH):
                    nc.tensor.matmul(
                        vps[:], xap(ci, t16 * P, (t16 + 1) * P),
                        wv_sb[:, ci, :],
                        start=(ci == 0), stop=(ci == NCH - 1),
                    )
                nc.vector.tensor_copy(
                    v_sb[:, t16, :, 0:D],
                    vps[:].rearrange("p (h x) -> p h x", h=HL),
                )

            ych_tiles = {}
            yt_tiles = {}

            def scores(blk, h, pr):
                """Scores + exp (+ causal mask) for one k-tile pair."""
                q0, qw, nkt = BLOCKS[blk]
                mt, hh = h // 2, h % 2
                rsl = slice(hh * D, (hh + 1) * D)
                st = ps.tile([P, 2 * qw], F32, tag="st", bufs=3, name="st")
                pt = sb.tile([P, 2 * qw], BF, tag="pt", bufs=5, name="pt")
                diag = []
                for half in range(2):
                    kt = 2 * pr + half
                    qv = max(kt * P - q0, 0)
                    nc.tensor.matmul(
                        st[:, half * qw + qv:(half + 1) * qw],
                        kT_sb[rsl, mt, kt * P:(kt + 1) * P],
                        qT_sb[rsl, mt, q0 + qv:q0 + qw],
                        start=True, stop=True,
                    )
                    if kt * P >= q0:
                        diag.append(half * qw + qv)
                qv0 = max(2 * pr * P - q0, 0)
                nc.scalar.activation(
                    pt[:, qv0:], st[:, qv0:], AF.Exp, scale=float(SCALE)
                )
                for c0 in diag:
                    nc.vector.tensor_mul(
                        pt[:, c0:c0 + P], pt[:, c0:c0 + P], tri[:]
                    )
                return pt

            def pv(blk, h, pr, pt):
                q0, qw, nkt = BLOCKS[blk]
                yt = yt_tiles.get((blk, h))
                if yt is None:
                    yt = yt_tiles[(blk, h)] = ps.tile(
                        [D + 1, qw], F32, tag="yt", bufs=2, name="yt"
                    )
                for half in range(2):
                    kt = 2 * pr + half
                    qv = max(kt * P - q0, 0)
                    nc.tensor.matmul(
                        yt[:, qv:qw],
                        v_sb[:, kt, h, :],
                        pt[:, half * qw + qv:(half + 1) * qw],
                        start=(kt == 0), stop=(kt == nkt - 1),
                    )

            def evict(blk, h):
                q0, qw, nkt = BLOCKS[blk]
                yt = yt_tiles.pop((blk, h))
                den = sb.tile([1, qw], F32, tag="den", bufs=4, name="den")
                nc.vector.tensor_copy(den[:], yt[D:D + 1, :])
                bc = sb.tile([D, qw], F32, tag="bc", bufs=3, name="bc")
                nc.gpsimd.partition_broadcast(bc[:], den[:])
                rcp = sb.tile([D, qw], F32, tag="rcp", bufs=3, name="rcp")
                scr = sb.tile([D, qw], F32, tag="scr", bufs=3, name="scr")
                nc.vector.reciprocal_approx_accurate(
                    rcp[:], bc[:], scratch=scr[:]
                )
                mt, hh = h // 2, h % 2
                ych = ych_tiles[blk]
                nc.vector.tensor_mul(
                    ych[hh * D:(hh + 1) * D, mt, :], yt[0:D, :], rcp[:]
                )

            def po_group(c):
                """Partial O-proj for block c -> DRAM -> RS trigger."""
                q0, qw, nkt = BLOCKS[c]
                ych = ych_tiles.pop(c)
                ob = sb.tile([P, NRT, qw], BF, tag="ob", bufs=2, name="ob")
                for rt in range(NRT):
                    po = ps.tile([P, qw], F32, tag="st", bufs=3, name="po")
                    for ci in range(MT):
                        nc.tensor.matmul(
                            po[:], wo_sb[:, ci, rt * P:(rt + 1) * P],
                            ych[:, ci, :],
                            start=(ci == 0), stop=(ci == MT - 1),
                        )
                    nc.vector.tensor_copy(ob[:, rt, :], po[:])
                dview = po_dram[c][:].rearrange("(r p) t -> p r t", p=P)
                nc.sync.dma_start(dview, ob[:])
                nc.gpsimd.collective_compute(
                    "ReduceScatter",
                    mybir.AluOpType.add,
                    replica_groups=RGROUPS,
                    ins=[po_dram[c][:]],
                    outs=[rs_out[c][:]],
                )

            # ---------- job stream ----------
            # each job: (phase1, phase2) — phase2 runs LA jobs later
            jobs = []

            def add_filler(fn, *args):
                jobs.append((lambda a=args: fn(*a), None))

            def add_pair(blk, h, pr, last):
                def p1(a=(blk, h, pr)):
                    return scores(*a)

                def p2(pt, a=(blk, h, pr), last=last):
                    pv(*a, pt)
                    if last:
                        evict(a[0], a[1])
                jobs.append((p1, p2))

            # chunk 0 projections (immediate: block 0 needs them; solo
            # 512-wide so they only gate on the fast x_head DMAs)
            for wsb, dst in ((wq_sb, qT_sb), (wk_sb, kT_sb)):
                for mt in range(MT):
                    add_filler(proj_one, 0, wsb, mt, dst)
            for t16 in range(4):
                add_filler(v_tile, t16)

            # blocks with fillers woven in
            for blk, (q0, qw, nkt) in enumerate(BLOCKS):
                npr = nkt // 2
                for h in range(HL):
                    for pr in range(npr):
                        add_pair(blk, h, pr, pr == npr - 1)
                    # weave fillers after each head
                    if blk == 0:
                        if h == 0:
                            for wsb, dst in ((wq_sb, qT_sb), (wk_sb, kT_sb)):
                                for mt in range(MT):
                                    add_filler(proj_one, 1, wsb, mt, dst)
                        elif h == 1:
                            for t16 in range(4, 8):
                                add_filler(v_tile, t16)
                        elif h == 2:
                            for t16 in range(8, 12):
                                add_filler(v_tile, t16)
                        else:
                            for t16 in range(12, 16):
                                add_filler(v_tile, t16)
                    elif blk == 1:
                        if h == 0:
                            add_filler(po_group, 0)
                        elif h == 1:
                            for mt in range(MT):
                                add_filler(proj_pair, 1, wq_sb, mt, qT_sb)
                        elif h == 2:
                            for mt in range(MT):
                                add_filler(proj_pair, 1, wk_sb, mt, kT_sb)
                    elif blk > 1 and h == 0:
                        add_filler(po_group, blk - 1)
            # flush the pipeline (phase2 lags by LA) before the last po
            for _ in range(LA):
                add_filler(keepwarm, 1)
            add_filler(po_group, NBLK - 1)

            # ---------- software-pipelined emission ----------
            for blk, (q0, qw, nkt) in enumerate(BLOCKS):
                ych_tiles[blk] = sb.tile([P, MT, qw], BF, tag="ych",
                                         bufs=2, name="ych")

            pending = []
            for i in range(len(jobs) + LA):
                if i < len(jobs):
                    p1, p2 = jobs[i]
                    r = p1()
                    pending.append((p2, r))
                if i >= LA:
                    p2, r = pending[i - LA]
                    if p2 is not None:
                        p2(r)

            engs = [nc.sync, nc.gpsimd]
            for c, (q0, qw, nkt) in enumerate(BLOCKS):
                engs[c % 2].dma_start(out[:, q0:q0 + qw], rs_out[c][:])

    nc.finalize()
    return nc


# pv() needs yt allocated; allocate inside pv via yt_tiles guard
_GRAPH = None


def _get_graph():
    global _GRAPH
    if _GRAPH is None:
        _GRAPH = build_graph()
    return _GRAPH


def prepare_in_maps(x, Wq, Wk, Wv, Wo):
    x = np.asarray(x, np.float32)
    Wq = np.asarray(Wq, np.float32)
    Wk = np.asarray(Wk, np.float32)
    Wv = np.asarray(Wv, np.float32)
    Wo = np.asarray(Wo, np.float32)

    bf = ml_dtypes.bfloat16
    xTh = [np.ascontiguousarray(x[b].T).astype(bf) for b in range(B)]
    in_maps = []
    for r in range(N_CORES):
        b, g = r // GR, r % GR
        sl = slice(g * DL, (g + 1) * DL)
        wqkv = np.empty((P, 3, NCH, DL), np.float32)
        for w, W in enumerate((Wq, Wk, Wv)):
            wqkv[:, w] = W[sl].T.reshape(NCH, P, DL).transpose(1, 0, 2)
        wo = np.ascontiguousarray(Wo[:, sl].T)  # [DL, C] = lhsT
        woT = wo.reshape(MT, P, C).transpose(1, 0, 2)  # [p, ci, C]
        wall = np.concatenate(
            [wqkv.reshape(P, 3 * NCH * DL), woT.reshape(P, MT * C)], axis=1
        )
        in_maps.append({
            "xT": xTh[b],
            "wall": np.ascontiguousarray(wall).astype(bf),
        })
    return in_maps


def assemble_output(results):
    outT = np.empty((B, C, TL), np.float32)
    for r in range(N_CORES):
        b, g = r // GR, r % GR
        outT[b, g * DL:(g + 1) * DL] = np.asarray(
            results[r]["out"], np.float32
        )
    return np.ascontiguousarray(outT.transpose(0, 2, 1))  # [B, T, C]


def kernel(x, Wq, Wk, Wv, Wo):
    nc = _get_graph()
    in_maps = prepare_in_maps(x, Wq, Wk, Wv, Wo)
    res = run_bass_kernel_spmd(nc, in_maps, core_ids=list(range(N_CORES)))
    return assemble_output(res.results)
